# revision 16
# baseline (speedup 1.0000x reference)
"""Trainium2 Bass kernel for 3x3 same-padding conv (Winograd reference problem).

Strategy: data-parallel over batch across 8 NeuronCores (8 images/core).
Per core the conv is computed directly as 9 shifted fp32r matmuls (taps)
x 2 input-channel halves accumulated in PSUM:
    out[o, (h,w)] = sum_{c,u,v} w[o,c,u,v] * xp[c, h+u, w+v]
The padded input for a core (2 x 128 x 8 x 34 x 34 fp32) stays resident in
SBUF, so each of the 32 output tiles (128 out-ch x 512 pixels) runs its 18
matmuls back-to-back with no DMA waits (PE stays warm).
"""

import numpy as np

import concourse.bacc as bacc
import concourse.mybir as mybir
import concourse.tile as tile
from concourse.bass_utils import run_bass_kernel_spmd

B_FULL, C, O, H = 64, 256, 256, 32
N_CORES = 8
B_SH = B_FULL // N_CORES  # images per core
HP = H + 2  # padded spatial
CH = C // 128  # input-channel halves
OH = O // 128  # output-channel halves

_CACHE = {}


def _build():
    nc = bacc.Bacc(None, target_bir_lowering=False)
    f32 = mybir.dt.float32
    f32r = mybir.dt.float32r

    xp = nc.dram_tensor("xp", [CH, 128, B_SH, HP, HP], f32r, kind="ExternalInput")
    wt = nc.dram_tensor("wt", [CH, 128, 9, O], f32r, kind="ExternalInput")
    y = nc.dram_tensor("y", [B_SH, O, H, H], f32, kind="ExternalOutput")

    with tile.TileContext(nc) as tc:
        with (
            tc.tile_pool(name="xpool", bufs=1) as xpool,
            tc.tile_pool(name="wpool", bufs=1) as wpool,
            tc.tile_pool(name="opool", bufs=6) as opool,
            tc.tile_pool(name="psum", bufs=7, space="PSUM") as psum,
        ):
            # DMA issue order tuned for ramp-up: the first output tile only
            # needs x(b0,ch0) + w(ch0,uv0), so those go first and the rest
            # streams in behind the PE.
            w_sb = {}
            x_sb = {}

            def load_x(b, ch):
                # two row-half DMAs so hh=0 matmuls can start before the
                # bottom half of the image lands
                x_t = xpool.tile(
                    [128, HP, HP], f32r, tag=f"x{ch}_{b}", name=f"x{ch}_{b}"
                )
                nc.sync.dma_start(x_t[:, 0:18, :], xp[ch, :, b, 0:18])
                nc.sync.dma_start(x_t[:, 18:HP, :], xp[ch, :, b, 18:HP])
                x_sb[(ch, b)] = x_t

            def load_w(ch, uv):
                w_t = wpool.tile(
                    [128, O], f32r, tag=f"w{ch}_{uv}", name=f"w{ch}_{uv}"
                )
                nc.sync.dma_start(w_t[:], wt[ch, :, uv])
                w_sb[(ch, uv)] = w_t

            load_x(0, 0)
            load_w(0, 0)
            load_w(0, 1)
            load_x(0, 1)
            for uv in range(2, 9):
                load_w(0, uv)
            for uv in range(9):
                load_w(1, uv)
            for b in range(1, B_SH):
                for ch in range(CH):
                    load_x(b, ch)

            # Warm up the PE clock (HAM ramps to 2.4GHz after ~3.4us of
            # activity) during the initial DMA wait: dummy matmuls with no
            # DMA dependency.
            warm = xpool.tile([128, 512], mybir.dt.bfloat16, tag="warm", name="warm")
            nc.vector.memset(warm[:], 0.0)
            wacc = psum.tile([128, 512], f32, tag="wacc", name="wacc", bufs=1)
            for _ in range(6):
                nc.tensor.matmul(wacc[:], warm[:, 0:128], warm[:], start=True,
                                 stop=True)

            for b in range(B_SH):
                for hh in (0, 16):
                    for oh in range(OH):
                        acc = psum.tile([128, 16, H], f32)
                        k = 0
                        for ch in range(CH):
                            for u in range(3):
                                for v in range(3):
                                    nc.tensor.matmul(
                                        acc[:],
                                        w_sb[(ch, 3 * u + v)][
                                            :, oh * 128:(oh + 1) * 128
                                        ],
                                        x_sb[(ch, b)][:, hh + u:hh + u + 16, v:v + H],
                                        start=(k == 0),
                                        stop=(k == 17),
                                    )
                                    k += 1
                        o_t = opool.tile([128, 16, H], f32)
                        nc.vector.tensor_copy(o_t[:], acc[:])
                        nc.sync.dma_start(
                            y[b, oh * 128:(oh + 1) * 128, hh:hh + 16, :], o_t[:]
                        )
    nc.compile()
    return nc


def _ensure_ntff_hook():
    """Register the antenv.axon_hooks shim so trace=True can capture NTFFs."""
    import sys
    import types

    if "antenv.axon_hooks" in sys.modules:
        return
    try:
        from trn_agent_boot.trn_boot import _ntff_profile_via_ctypes

        hook = _ntff_profile_via_ctypes("/opt/axon/libaxon_pjrt.so")
    except Exception:
        hook = None
    mod = types.ModuleType("antenv.axon_hooks")
    mod.get_axon_ntff_profile_hook = lambda: hook
    mod.set_axon_ntff_profile_hook = lambda h: None
    sys.modules["antenv.axon_hooks"] = mod
    try:
        import antenv

        antenv.axon_hooks = mod
    except ImportError:
        pass


def run(x, weight, trace=False):
    """Returns (output, BassKernelResults)."""
    if trace:
        _ensure_ntff_hook()
    x = np.asarray(x, dtype=np.float32)
    weight = np.asarray(weight, dtype=np.float32)

    if "nc" not in _CACHE:
        _CACHE["nc"] = _build()
    nc = _CACHE["nc"]

    # (O, C, 3, 3) -> (CH, 128, 9, O)
    wt = np.ascontiguousarray(
        weight.transpose(1, 2, 3, 0).reshape(CH, 128, 9, O)
    )
    xpad = np.pad(x, ((0, 0), (0, 0), (1, 1), (1, 1)))  # (B, C, 34, 34)

    in_maps = []
    for i in range(N_CORES):
        xs = xpad[i * B_SH:(i + 1) * B_SH]  # (B_SH, C, 34, 34)
        xs = np.ascontiguousarray(
            xs.transpose(1, 0, 2, 3).reshape(CH, 128, B_SH, HP, HP)
        )
        in_maps.append({"xp": xs, "wt": wt})

    res = run_bass_kernel_spmd(
        nc, in_maps, core_ids=list(range(N_CORES)), trace=trace
    )
    out = np.concatenate([res.results[i]["y"] for i in range(N_CORES)], axis=0)
    return out, res


def kernel(x, weight, A_t=None, B_t=None, G=None, **_unused):
    return run(x, weight)[0]
